# revision 10
# baseline (speedup 1.0000x reference)
"""Trainium2 Bass kernel for nn_CentroidLayer (vq_codebook).

reference:
    q = l2norm(query_emb)  [B, D]
    c = l2norm(centroid_emb)  [P, D]
    sim = q @ c.T ; masked with active_mask (-1e9)
    weights = softmax(sim, -1)
    context = weights @ centroid_emb
    hard = argmax(sim_masked, -1)

B=131072, P=64, D=256, f32.  8 NeuronCores, batch-sharded (16384 rows/core).

Per-core pipeline (16 superblocks of 1024 rows):
  - load q row-tiles [128, 256]x8
  - norms: ACT Square (batched) -> DVE 3D-reduce -> ACT ln -> ACT exp(-0.5 ln)
  - q_hat = q * inv_norm (DVE tensor_scalar per tile)
  - PE transposes q_hat -> qT in PSUM -> evacuate to SBUF (ACT/DVE split)
  - sim^T [p, b] = cT (stationary, host-normalized) @ qT, fp32, col-packed
    halves on partitions 0:64 / 64:128 of one PSUM bank
  - exp(sim + mask_bias) via one ACT op (bias is per-partition AP)
  - ctx matmul per 128-row chunk: lhsT = expsimT slice [64,128],
    rhs = [C | ones | I64] [64, 321] -> psum [context~ | denom | expsim-row]
  - reciprocal(denom), fused evacuate+normalize (tensor_scalar from PSUM)
  - argmax over the row-layout expsim columns (reduce/is_ge/mul/reduce)
  - hard indices accumulated [128, 16*8], transposed once at the end via PE,
    converted to int32 (63 - x) and stored contiguously.

Assumes at least one active centroid (harness uses all-ones mask).
"""
import sys
import numpy as np

sys.path.insert(0, "/opt/trn_rl_repo")

import concourse.bass as bass
import concourse.bacc as bacc
import concourse.mybir as mybir
import concourse.tile as tile
from concourse.bass_utils import run_bass_kernel_spmd

from concourse.hw_specs import get_activation_tables


class BaccOneSet(bacc.Bacc):
    """Force all activations onto the natural_log_exp_and_others table set
    (contains Ln, Exp, Square, Copy, Identity) so only one ACT_TABLE_LOAD is
    emitted instead of per-function set thrashing (~2.7us per switch)."""

    _SHARED = {"Exp", "Ln", "Square", "Copy", "Identity"}

    def insert_act_table_loads(self):
        from concourse import inst_simplify  # noqa: F401  (parity w/ bacc)
        has_activation = any(
            isinstance(i, mybir.InstActivation)
            for b in self.main_func.blocks
            for i in b.instructions
        )
        if not has_activation:
            return
        tables = list(get_activation_tables(self.m.arch).items())
        filtered = []
        for name, funcs in tables:
            if name == "natural_log_exp_and_others":
                filtered.append((name, set(funcs)))
            else:
                filtered.append((name, {f for f in funcs
                                        if f.name not in self._SHARED}))
        import bass_rust as _bass_rust
        _bass_rust.insert_act_table_loads(self, filtered)


F32 = mybir.dt.float32
F32R = mybir.dt.float32r
I32 = mybir.dt.int32
AF = mybir.ActivationFunctionType
ALU = mybir.AluOpType
AX = mybir.AxisListType

B, P, D = 131072, 64, 256
NCORES = 8
B_LOC = B // NCORES          # 16384
SB = 1024                    # rows per superblock
NSB = B_LOC // SB            # 16

# consts layout (free-dim columns of the [128, 642] f32 consts tensor)
CT0 = 0        # cT chunk d0:128   [128, 64]
CT1 = 64       # cT chunk d128:256 [128, 64]
CP = 128       # C' = [C | ones | I64 | pad] duplicated rows -> [128, 322]
MB = 450       # mask bias [128, 1]
REV = 451      # rev iota 63-j [128, 64]
EYE = 515      # eye128 [128, 128]
CW = 643

USE_F32R_CTX = True   # ctx matmul in fp32r (1 cy/row) vs fp32 (4 cy/row)


def build_nc(b_loc=B_LOC, use_f32r=USE_F32R_CTX):
    nsb = b_loc // SB
    nc = BaccOneSet("TRN2", target_bir_lowering=False, debug=False,
                   num_devices=NCORES)

    q_d = nc.dram_tensor("q", [b_loc, D], F32, kind="ExternalInput")
    consts_d = nc.dram_tensor("consts", [128, CW], F32, kind="ExternalInput")
    if use_f32r:
        constsr_d = nc.dram_tensor("constsr", [128, 258], F32R,
                                   kind="ExternalInput")
    ctx_d = nc.dram_tensor("context", [b_loc, D], F32, kind="ExternalOutput")
    w_d = nc.dram_tensor("weights", [b_loc, P], F32, kind="ExternalOutput")
    hard_d = nc.dram_tensor("hard", [b_loc], I32, kind="ExternalOutput")

    EXPT = F32R if use_f32r else F32

    with tile.TileContext(nc) as tc:
        with tc.tile_pool(name="constp", bufs=1) as constp, \
             tc.tile_pool(name="qp", bufs=3) as qp, \
             tc.tile_pool(name="sqp", bufs=3) as sqp, \
             tc.tile_pool(name="qtp", bufs=6) as qtp, \
             tc.tile_pool(name="expp", bufs=2) as expp, \
             tc.tile_pool(name="stripp", bufs=2) as stripp, \
             tc.tile_pool(name="smallp", bufs=3) as smallp, \
             tc.tile_pool(name="eqp", bufs=2) as eqp, \
             tc.tile_pool(name="hardp", bufs=1) as hardp, \
             tc.tile_pool(name="psqt", bufs=2, space="PSUM") as psqt, \
             tc.tile_pool(name="psim", bufs=2, space="PSUM") as psim, \
             tc.tile_pool(name="pctx", bufs=2, space="PSUM") as pctx:

            consts = constp.tile([128, CW], F32)
            nc.sync.dma_start(consts[:], consts_d[:])
            if use_f32r:
                constsr = constp.tile([128, 258], F32R)
                nc.sync.dma_start(constsr[:], constsr_d[:])
                cp_ap = constsr[:, 0:258]
            else:
                cp_ap = consts[:, CP:CP + 258]
            eye = consts[:, EYE:EYE + 128]

            hardacc = hardp.tile([128, nsb * 8], F32)

            for s in range(nsb):
                b0 = SB * s
                # ---- load q: 8 tiles of [128, 256] -----------------------
                q_sb = qp.tile([128, 8 * D], F32, tag="qsb")
                for t in range(8):
                    nc.sync.dma_start(q_sb[:, D * t:D * (t + 1)],
                                      q_d[b0 + 128 * t:b0 + 128 * (t + 1), :])

                # ---- norms ----------------------------------------------
                sq = sqp.tile([128, 8 * D], F32, tag="sq")
                nc.scalar.activation(sq[:], q_sb[:], AF.Square)
                ss = smallp.tile([128, 8], F32, tag="ss")
                nc.vector.tensor_reduce(
                    ss[:], sq[:].rearrange("p (t d) -> p t d", d=D),
                    axis=AX.X, op=ALU.add)
                lns = smallp.tile([128, 8], F32, tag="lns")
                nc.scalar.activation(lns[:], ss[:], AF.Ln)
                inv = smallp.tile([128, 8], F32, tag="inv")
                nc.scalar.activation(inv[:], lns[:], AF.Exp, scale=-0.5)

                # ---- q_hat = q * inv (in place) -------------------------
                for t in range(8):
                    nc.gpsimd.tensor_scalar_mul(
                        q_sb[:, D * t:D * (t + 1)],
                        q_sb[:, D * t:D * (t + 1)],
                        inv[:, t:t + 1])

                # ---- transpose to qT, per (half, d-chunk) ---------------
                qts = {}
                for h in range(2):
                    for dc in range(2):
                        pt = psqt.tile([128, 512], F32, tag="pqt")
                        for t4 in range(4):
                            t = 4 * h + t4
                            nc.tensor.transpose(
                                pt[:, 128 * t4:128 * (t4 + 1)],
                                q_sb[:, D * t + 128 * dc:D * t + 128 * dc + 128],
                                eye)
                        qt = qtp.tile([128, 512], F32, tag="qt")
                        nc.scalar.copy(qt[:], pt[:])
                        qts[(h, dc)] = qt

                # ---- sim^T: [p(2 halves packed), 512] fp32 --------------
                ps = psim.tile([128, 512], F32, tag="ps")
                for h in range(2):
                    for dc in range(2):
                        nc.tensor.matmul(
                            ps[64 * h:64 * h + 64, :],
                            consts[:, 64 * dc:64 * dc + 64],
                            qts[(h, dc)][:],
                            start=(dc == 0), stop=(dc == 1))

                # ---- exp(sim + mask_bias) -------------------------------
                expT = expp.tile([128, 512], F32, tag="expT")
                nc.scalar.activation(expT[:], ps[:], AF.Exp,
                                     bias=consts[:, MB:MB + 1], scale=1.0)
                if use_f32r:
                    expTr = expp.tile([128, 512], F32R, tag="expTr")
                    nc.vector.tensor_copy(expTr[:], expT[:])
                else:
                    expTr = expT

                # ---- ctx matmuls + fused evac/normalize -----------------
                strip = stripp.tile([128, 8 * 322], F32, tag="strip")
                for p4 in range(4):
                    pc = pctx.tile([128, 1024], F32, tag="pc")
                    for k in range(2):
                        j = 2 * p4 + k
                        h, c = j // 4, j % 4
                        nc.tensor.matmul(
                            pc[:, 512 * k:512 * k + 258],
                            expTr[64 * h:64 * h + 64, 128 * c:128 * (c + 1)],
                            cp_ap[64 * h:64 * h + 64, :],
                            start=True, stop=True)
                        nc.tensor.matmul(
                            pc[:, 512 * k + 257:512 * k + 322],
                            expT[64 * h:64 * h + 64, 128 * c:128 * (c + 1)],
                            consts[64 * h:64 * h + 64, CP + 256:CP + 321],
                            start=True, stop=True)
                    dn = smallp.tile([128, 2], F32, tag="dn")
                    nc.vector.tensor_copy(
                        dn[:], pc[:].rearrange("p (k n) -> p k n", n=512)[:, :, 257:258])
                    rc = smallp.tile([128, 2], F32, tag="rc")
                    nc.vector.reciprocal(rc[:], dn[:])
                    for k in range(2):
                        j = 2 * p4 + k
                        if j % 4 == 0:
                            nc.vector.tensor_scalar_mul(
                                strip[:, 322 * j:322 * (j + 1)],
                                pc[:, 512 * k:512 * k + 322],
                                rc[:, k:k + 1])
                        else:
                            nc.scalar.activation(
                                strip[:, 322 * j:322 * (j + 1)],
                                pc[:, 512 * k:512 * k + 322],
                                AF.Copy, scale=rc[:, k:k + 1])

                # ---- outputs: context / weights -------------------------
                nc.sync.dma_start(
                    ctx_d[b0:b0 + SB, :].rearrange("(j p) d -> p j d", p=128),
                    strip[:].rearrange("p (j n) -> p j n", n=322)[:, :, 0:256])
                nc.sync.dma_start(
                    w_d[b0:b0 + SB, :].rearrange("(j p) d -> p j d", p=128),
                    strip[:].rearrange("p (j n) -> p j n", n=322)[:, :, 258:322])

                # ---- argmax ---------------------------------------------
                wview = strip[:].rearrange("p (j n) -> p j n", n=322)[:, :, 258:322]
                rm = smallp.tile([128, 8], F32, tag="rm")
                nc.vector.tensor_reduce(rm[:], wview, axis=AX.X, op=ALU.max)
                eq = eqp.tile([128, 512], F32, tag="eq")
                eqv = eq[:].rearrange("p (j n) -> p j n", n=64)
                nc.vector.tensor_tensor(
                    eqv, wview,
                    rm[:].rearrange("p (j o) -> p j o", o=1).broadcast_to([128, 8, 64]),
                    op=ALU.is_ge)
                nc.vector.tensor_tensor(
                    eqv, eqv,
                    consts[:, REV:REV + 64].rearrange("p (o n) -> p o n", o=1)
                        .broadcast_to([128, 8, 64]),
                    op=ALU.mult)
                nc.vector.tensor_reduce(
                    hardacc[:, 8 * s:8 * (s + 1)], eqv, axis=AX.X, op=ALU.max)

            # ---- finish hard: transpose, 63-x, cast, store --------------
            ktot = nsb * 8
            for g in range((ktot + 127) // 128):
                cols = min(128, ktot - 128 * g)
                pt = psqt.tile([128, 512], F32, tag="pqt")
                nc.tensor.transpose(
                    pt[0:cols, 0:128], hardacc[:, 128 * g:128 * g + cols], eye)
                hi = smallp.tile([128, 128], I32, tag="hi")
                nc.vector.tensor_scalar(
                    hi[0:cols, :], pt[0:cols, 0:128], -1.0, 63.0,
                    ALU.mult, ALU.add)
                nc.sync.dma_start(
                    hard_d[128 * 128 * g:128 * (128 * g + cols)]
                        .rearrange("(k p) -> k p", p=128),
                    hi[0:cols, :])

    nc.finalize()
    return nc


def _host_consts(centroid_emb, active_mask):
    c = np.asarray(centroid_emb, dtype=np.float32)
    mask = np.asarray(active_mask).astype(bool)
    c64 = c.astype(np.float64)
    cn = (c64 / np.maximum(np.linalg.norm(c64, axis=1, keepdims=True), 1e-12)
          ).astype(np.float32)                      # [P, D] normalized
    cT = cn.T                                       # [D, P] = [256, 64]
    consts = np.zeros((128, CW), dtype=np.float32)
    consts[:, CT0:CT0 + 64] = cT[0:128]
    consts[:, CT1:CT1 + 64] = cT[128:256]
    cp = np.concatenate(
        [c, np.ones((P, 1), np.float32), np.eye(P, dtype=np.float32),
         np.zeros((P, 1), np.float32)], axis=1)
    consts[0:64, CP:CP + 322] = cp
    consts[64:128, CP:CP + 322] = cp
    mb = np.where(mask, 0.0, -1e9).astype(np.float32)
    consts[0:64, MB] = mb
    consts[64:128, MB] = mb
    consts[:, REV:REV + 64] = (63.0 - np.arange(64, dtype=np.float32))[None, :]
    consts[:, EYE:EYE + 128] = np.eye(128, dtype=np.float32)
    cpr = np.concatenate([c, np.ones((P, 1), np.float32),
                      np.zeros((P, 1), np.float32)], axis=1)
    cpr2 = np.zeros((128, 258), np.float32)
    cpr2[0:64] = cpr
    cpr2[64:128] = cpr
    return consts, cpr2


_NC_CACHE = {}


def _get_nc(b_loc, use_f32r):
    key = (b_loc, use_f32r)
    if key not in _NC_CACHE:
        _NC_CACHE[key] = build_nc(b_loc, use_f32r)
    return _NC_CACHE[key]


def run(query_emb, centroid_emb, active_mask, use_f32r=USE_F32R_CTX,
        trace=False):
    q = np.ascontiguousarray(np.asarray(query_emb, dtype=np.float32))
    b = q.shape[0]
    b_loc = b // NCORES
    nc = _get_nc(b_loc, use_f32r)
    consts, cp = _host_consts(centroid_emb, active_mask)
    shards = q.reshape(NCORES, b_loc, D)
    in_maps = []
    for i in range(NCORES):
        m = {"q": shards[i], "consts": consts}
        if use_f32r:
            m["constsr"] = cp
        in_maps.append(m)
    res = run_bass_kernel_spmd(nc, in_maps, list(range(NCORES)), trace=trace)
    ctx = np.concatenate([r["context"] for r in res.results], axis=0)
    w = np.concatenate([r["weights"] for r in res.results], axis=0)
    hard = np.concatenate([r["hard"] for r in res.results], axis=0)
    return (ctx, w, hard), res


def kernel(query_emb, centroid_emb, active_mask):
    (ctx, w, hard), _ = run(query_emb, centroid_emb, active_mask)
    return ctx, w, hard.astype(np.int32)


# revision 11
# speedup vs baseline: 1.3674x; 1.3674x over previous
"""Trainium2 Bass kernel for nn_CentroidLayer (vq_codebook).

reference:
    q = l2norm(query_emb)  [B, D]
    c = l2norm(centroid_emb)  [P, D]
    sim = q @ c.T ; masked with active_mask (-1e9)
    weights = softmax(sim, -1)
    context = weights @ centroid_emb
    hard = argmax(sim_masked, -1)

B=131072, P=64, D=256, f32.  8 NeuronCores, batch-sharded (16384 rows/core).

Per-core pipeline (16 superblocks of 1024 rows):
  - load q row-tiles [128, 256]x8
  - norms: ACT Square (batched) -> DVE 3D-reduce -> ACT ln -> ACT exp(-0.5 ln)
  - q_hat = q * inv_norm (DVE tensor_scalar per tile)
  - PE transposes q_hat -> qT in PSUM -> evacuate to SBUF (ACT/DVE split)
  - sim^T [p, b] = cT (stationary, host-normalized) @ qT, fp32, col-packed
    halves on partitions 0:64 / 64:128 of one PSUM bank
  - exp(sim + mask_bias) via one ACT op (bias is per-partition AP)
  - ctx matmul per 128-row chunk: lhsT = expsimT slice [64,128],
    rhs = [C | ones | I64] [64, 321] -> psum [context~ | denom | expsim-row]
  - reciprocal(denom), fused evacuate+normalize (tensor_scalar from PSUM)
  - argmax over the row-layout expsim columns (reduce/is_ge/mul/reduce)
  - hard indices accumulated [128, 16*8], transposed once at the end via PE,
    converted to int32 (63 - x) and stored contiguously.

Assumes at least one active centroid (harness uses all-ones mask).
"""
import sys
import numpy as np

sys.path.insert(0, "/opt/trn_rl_repo")

import concourse.bass as bass
import concourse.bacc as bacc
import concourse.mybir as mybir
import concourse.tile as tile
from concourse.bass_utils import run_bass_kernel_spmd

from concourse.hw_specs import get_activation_tables


class BaccOneSet(bacc.Bacc):
    """Force all activations onto the natural_log_exp_and_others table set
    (contains Ln, Exp, Square, Copy, Identity) so only one ACT_TABLE_LOAD is
    emitted instead of per-function set thrashing (~2.7us per switch)."""

    _SHARED = {"Exp", "Ln", "Square", "Copy", "Identity"}

    def insert_act_table_loads(self):
        from concourse import inst_simplify  # noqa: F401  (parity w/ bacc)
        has_activation = any(
            isinstance(i, mybir.InstActivation)
            for b in self.main_func.blocks
            for i in b.instructions
        )
        if not has_activation:
            return
        tables = list(get_activation_tables(self.m.arch).items())
        filtered = []
        for name, funcs in tables:
            if name == "natural_log_exp_and_others":
                filtered.append((name, set(funcs)))
            else:
                filtered.append((name, {f for f in funcs
                                        if f.name not in self._SHARED}))
        import bass_rust as _bass_rust
        _bass_rust.insert_act_table_loads(self, filtered)


F32 = mybir.dt.float32
F32R = mybir.dt.float32r
I32 = mybir.dt.int32
AF = mybir.ActivationFunctionType
ALU = mybir.AluOpType
AX = mybir.AxisListType

B, P, D = 131072, 64, 256
NCORES = 8
B_LOC = B // NCORES          # 16384
SB = 1024                    # rows per superblock
NSB = B_LOC // SB            # 16

# consts layout (free-dim columns of the [128, 642] f32 consts tensor)
CT0 = 0        # cT chunk d0:128   [128, 64]
CT1 = 64       # cT chunk d128:256 [128, 64]
CP = 128       # C' = [C | ones | I64 | pad] duplicated rows -> [128, 322]
MB = 450       # mask bias [128, 1]
REV = 451      # rev iota 63-j [128, 64]
EYE = 515      # eye128 [128, 128]
CW = 643

USE_F32R_CTX = True   # ctx matmul in fp32r (1 cy/row) vs fp32 (4 cy/row)


def build_nc(b_loc=B_LOC, use_f32r=USE_F32R_CTX):
    nsb = b_loc // SB
    nc = BaccOneSet("TRN2", target_bir_lowering=False, debug=False,
                   num_devices=NCORES)

    q_d = nc.dram_tensor("q", [b_loc, D], F32, kind="ExternalInput")
    consts_d = nc.dram_tensor("consts", [128, CW], F32, kind="ExternalInput")
    if use_f32r:
        constsr_d = nc.dram_tensor("constsr", [128, 258], F32R,
                                   kind="ExternalInput")
    ctx_d = nc.dram_tensor("context", [b_loc, D], F32, kind="ExternalOutput")
    w_d = nc.dram_tensor("weights", [b_loc, P], F32, kind="ExternalOutput")
    hard_d = nc.dram_tensor("hard", [b_loc], I32, kind="ExternalOutput")

    EXPT = F32R if use_f32r else F32

    with tile.TileContext(nc) as tc:
        with tc.tile_pool(name="constp", bufs=1) as constp, \
             tc.tile_pool(name="qp", bufs=6) as qp, \
             tc.tile_pool(name="sqp", bufs=6) as sqp, \
             tc.tile_pool(name="qtp", bufs=6) as qtp, \
             tc.tile_pool(name="expp", bufs=4) as expp, \
             tc.tile_pool(name="stripp", bufs=3) as stripp, \
             tc.tile_pool(name="smallp", bufs=8) as smallp, \
             tc.tile_pool(name="eqp", bufs=3) as eqp, \
             tc.tile_pool(name="hardp", bufs=1) as hardp, \
             tc.tile_pool(name="psqt", bufs=2, space="PSUM") as psqt, \
             tc.tile_pool(name="psim", bufs=2, space="PSUM") as psim, \
             tc.tile_pool(name="pctx", bufs=2, space="PSUM") as pctx:

            consts = constp.tile([128, CW], F32)
            nc.sync.dma_start(consts[:], consts_d[:])
            if use_f32r:
                constsr = constp.tile([128, 258], F32R)
                nc.sync.dma_start(constsr[:], constsr_d[:])
                cp_ap = constsr[:, 0:258]
            else:
                cp_ap = consts[:, CP:CP + 258]
            eye = consts[:, EYE:EYE + 128]

            hardacc = hardp.tile([128, nsb * 8], F32)

            for s in range(nsb):
                b0 = SB * s
                # ---- load q: 8 tiles of [128, 256] -----------------------
                q_sb = qp.tile([128, 8 * D], F32, tag="qsb")
                for t in range(8):
                    nc.sync.dma_start(q_sb[:, D * t:D * (t + 1)],
                                      q_d[b0 + 128 * t:b0 + 128 * (t + 1), :])

                # ---- norms ----------------------------------------------
                sq = sqp.tile([128, 8 * D], F32, tag="sq")
                nc.scalar.activation(sq[:], q_sb[:], AF.Square)
                ss = smallp.tile([128, 8], F32, tag="ss")
                nc.vector.tensor_reduce(
                    ss[:], sq[:].rearrange("p (t d) -> p t d", d=D),
                    axis=AX.X, op=ALU.add)
                lns = smallp.tile([128, 8], F32, tag="lns")
                nc.scalar.activation(lns[:], ss[:], AF.Ln)
                inv = smallp.tile([128, 8], F32, tag="inv")
                nc.scalar.activation(inv[:], lns[:], AF.Exp, scale=-0.5)

                # ---- q_hat = q * inv (in place) -------------------------
                for t in range(8):
                    nc.gpsimd.tensor_scalar_mul(
                        q_sb[:, D * t:D * (t + 1)],
                        q_sb[:, D * t:D * (t + 1)],
                        inv[:, t:t + 1])

                # ---- transpose to qT, per (half, d-chunk) ---------------
                qts = {}
                for h in range(2):
                    for dc in range(2):
                        pt = psqt.tile([128, 512], F32, tag="pqt")
                        for t4 in range(4):
                            t = 4 * h + t4
                            nc.tensor.transpose(
                                pt[:, 128 * t4:128 * (t4 + 1)],
                                q_sb[:, D * t + 128 * dc:D * t + 128 * dc + 128],
                                eye)
                        qt = qtp.tile([128, 512], F32, tag="qt")
                        nc.scalar.copy(qt[:], pt[:])
                        qts[(h, dc)] = qt

                # ---- sim^T: [p(2 halves packed), 512] fp32 --------------
                ps = psim.tile([128, 512], F32, tag="ps")
                for h in range(2):
                    for dc in range(2):
                        nc.tensor.matmul(
                            ps[64 * h:64 * h + 64, :],
                            consts[:, 64 * dc:64 * dc + 64],
                            qts[(h, dc)][:],
                            start=(dc == 0), stop=(dc == 1))

                # ---- exp(sim + mask_bias) -------------------------------
                expT = expp.tile([128, 512], F32, tag="expT")
                nc.scalar.activation(expT[:], ps[:], AF.Exp,
                                     bias=consts[:, MB:MB + 1], scale=1.0)
                if use_f32r:
                    expTr = expp.tile([128, 512], F32R, tag="expTr")
                    nc.vector.tensor_copy(expTr[:], expT[:])
                else:
                    expTr = expT

                # ---- ctx matmuls + fused evac/normalize -----------------
                strip = stripp.tile([128, 8 * 322], F32, tag="strip")
                for p4 in range(4):
                    pc = pctx.tile([128, 1024], F32, tag="pc")
                    for k in range(2):
                        j = 2 * p4 + k
                        h, c = j // 4, j % 4
                        nc.tensor.matmul(
                            pc[:, 512 * k:512 * k + 258],
                            expTr[64 * h:64 * h + 64, 128 * c:128 * (c + 1)],
                            cp_ap[64 * h:64 * h + 64, :],
                            start=True, stop=True)
                        nc.tensor.matmul(
                            pc[:, 512 * k + 257:512 * k + 322],
                            expT[64 * h:64 * h + 64, 128 * c:128 * (c + 1)],
                            consts[64 * h:64 * h + 64, CP + 256:CP + 321],
                            start=True, stop=True)
                    dn = smallp.tile([128, 2], F32, tag="dn")
                    nc.vector.tensor_copy(
                        dn[:], pc[:].rearrange("p (k n) -> p k n", n=512)[:, :, 257:258])
                    rc = smallp.tile([128, 2], F32, tag="rc")
                    nc.vector.reciprocal(rc[:], dn[:])
                    for k in range(2):
                        j = 2 * p4 + k
                        if j % 4 == 0:
                            nc.vector.tensor_scalar_mul(
                                strip[:, 322 * j:322 * (j + 1)],
                                pc[:, 512 * k:512 * k + 322],
                                rc[:, k:k + 1])
                        else:
                            nc.scalar.activation(
                                strip[:, 322 * j:322 * (j + 1)],
                                pc[:, 512 * k:512 * k + 322],
                                AF.Copy, scale=rc[:, k:k + 1])

                # ---- outputs: context / weights -------------------------
                nc.sync.dma_start(
                    ctx_d[b0:b0 + SB, :].rearrange("(j p) d -> p j d", p=128),
                    strip[:].rearrange("p (j n) -> p j n", n=322)[:, :, 0:256])
                nc.sync.dma_start(
                    w_d[b0:b0 + SB, :].rearrange("(j p) d -> p j d", p=128),
                    strip[:].rearrange("p (j n) -> p j n", n=322)[:, :, 258:322])

                # ---- argmax ---------------------------------------------
                wview = strip[:].rearrange("p (j n) -> p j n", n=322)[:, :, 258:322]
                rm = smallp.tile([128, 8], F32, tag="rm")
                nc.vector.tensor_reduce(rm[:], wview, axis=AX.X, op=ALU.max)
                eq = eqp.tile([128, 512], F32, tag="eq")
                eqv = eq[:].rearrange("p (j n) -> p j n", n=64)
                nc.vector.tensor_tensor(
                    eqv, wview,
                    rm[:].rearrange("p (j o) -> p j o", o=1).broadcast_to([128, 8, 64]),
                    op=ALU.is_ge)
                nc.vector.tensor_tensor(
                    eqv, eqv,
                    consts[:, REV:REV + 64].rearrange("p (o n) -> p o n", o=1)
                        .broadcast_to([128, 8, 64]),
                    op=ALU.mult)
                nc.vector.tensor_reduce(
                    hardacc[:, 8 * s:8 * (s + 1)], eqv, axis=AX.X, op=ALU.max)

            # ---- finish hard: transpose, 63-x, cast, store --------------
            ktot = nsb * 8
            for g in range((ktot + 127) // 128):
                cols = min(128, ktot - 128 * g)
                pt = psqt.tile([128, 512], F32, tag="pqt")
                nc.tensor.transpose(
                    pt[0:cols, 0:128], hardacc[:, 128 * g:128 * g + cols], eye)
                hi = smallp.tile([128, 128], I32, tag="hi")
                nc.vector.tensor_scalar(
                    hi[0:cols, :], pt[0:cols, 0:128], -1.0, 63.0,
                    ALU.mult, ALU.add)
                nc.sync.dma_start(
                    hard_d[128 * 128 * g:128 * (128 * g + cols)]
                        .rearrange("(k p) -> k p", p=128),
                    hi[0:cols, :])

    nc.finalize()
    return nc


def _host_consts(centroid_emb, active_mask):
    c = np.asarray(centroid_emb, dtype=np.float32)
    mask = np.asarray(active_mask).astype(bool)
    c64 = c.astype(np.float64)
    cn = (c64 / np.maximum(np.linalg.norm(c64, axis=1, keepdims=True), 1e-12)
          ).astype(np.float32)                      # [P, D] normalized
    cT = cn.T                                       # [D, P] = [256, 64]
    consts = np.zeros((128, CW), dtype=np.float32)
    consts[:, CT0:CT0 + 64] = cT[0:128]
    consts[:, CT1:CT1 + 64] = cT[128:256]
    cp = np.concatenate(
        [c, np.ones((P, 1), np.float32), np.eye(P, dtype=np.float32),
         np.zeros((P, 1), np.float32)], axis=1)
    consts[0:64, CP:CP + 322] = cp
    consts[64:128, CP:CP + 322] = cp
    mb = np.where(mask, 0.0, -1e9).astype(np.float32)
    consts[0:64, MB] = mb
    consts[64:128, MB] = mb
    consts[:, REV:REV + 64] = (63.0 - np.arange(64, dtype=np.float32))[None, :]
    consts[:, EYE:EYE + 128] = np.eye(128, dtype=np.float32)
    cpr = np.concatenate([c, np.ones((P, 1), np.float32),
                      np.zeros((P, 1), np.float32)], axis=1)
    cpr2 = np.zeros((128, 258), np.float32)
    cpr2[0:64] = cpr
    cpr2[64:128] = cpr
    return consts, cpr2


_NC_CACHE = {}


def _get_nc(b_loc, use_f32r):
    key = (b_loc, use_f32r)
    if key not in _NC_CACHE:
        _NC_CACHE[key] = build_nc(b_loc, use_f32r)
    return _NC_CACHE[key]


def run(query_emb, centroid_emb, active_mask, use_f32r=USE_F32R_CTX,
        trace=False):
    q = np.ascontiguousarray(np.asarray(query_emb, dtype=np.float32))
    b = q.shape[0]
    b_loc = b // NCORES
    nc = _get_nc(b_loc, use_f32r)
    consts, cp = _host_consts(centroid_emb, active_mask)
    shards = q.reshape(NCORES, b_loc, D)
    in_maps = []
    for i in range(NCORES):
        m = {"q": shards[i], "consts": consts}
        if use_f32r:
            m["constsr"] = cp
        in_maps.append(m)
    res = run_bass_kernel_spmd(nc, in_maps, list(range(NCORES)), trace=trace)
    ctx = np.concatenate([r["context"] for r in res.results], axis=0)
    w = np.concatenate([r["weights"] for r in res.results], axis=0)
    hard = np.concatenate([r["hard"] for r in res.results], axis=0)
    return (ctx, w, hard), res


def kernel(query_emb, centroid_emb, active_mask):
    (ctx, w, hard), _ = run(query_emb, centroid_emb, active_mask)
    return ctx, w, hard.astype(np.int32)
